# revision 1
# baseline (speedup 1.0000x reference)
"""DeepFM forward on Trainium2 — 8-core data-parallel over batch.

Strategy:
  - Host packs sec_tables [F,V,16] + fst_tables [F,V,1] into one combined
    table [F*V, 17] so a single 68B-row indirect-DMA gather fetches both the
    second-order embedding row and the first-order scalar for each (b, f).
  - Each core handles 512 batch rows (4 groups of 128 partitions).
    Row-major gathered layout [128, (g f) * 17] feeds:
      * FM + first-order path: vector-engine reduces over f.
      * DNN path: PE transposes to [FD, batch] then 2-layer MLP with
        PSUM accumulation; final sum over H via ones-matmul.
  - Host combines out_a (fst+fm, [128,4]) + out_b (deep sum, [1,512]) + bias.
"""

import numpy as np

import concourse.bass as bass
import concourse.mybir as mybir
import concourse.tile as tile
from concourse import bacc
from concourse.bass_utils import run_bass_kernel_spmd

B, F, V, D, H = 4096, 39, 100000, 16, 400
NCORES = 8
BS = B // NCORES            # 512 batch rows per core
P = 128
G = BS // P                 # 4 groups of 128 rows
DP = D + 1                  # 17 floats gathered per (b, f)
FD = F * D                  # 624
KP = 5                      # k-chunks of 128 for layer-1 contraction
FDP = KP * P                # 640 (zero-padded contraction dim)
MCH = (128, 128, 128, 16)   # H=400 split into PSUM partition chunks
F32 = mybir.dt.float32

# matmul operand mode: "f32" (exact), "f32r" (1 cyc/row, reduced-precision
# multiplies; needs pre-rounded operands — not used), "bf16" (cast inputs
# to bf16 inside existing copies; matmuls run 1 cyc/row)
MM_MODE = "bf16"

_CACHE = {}


def _mmdt(mode):
    return {"f32": F32, "f32r": mybir.dt.float32r, "bf16": mybir.dt.bfloat16}[mode]


def _wdt(mode):
    # dtype weight/activation tiles are *stored* in
    return mybir.dt.bfloat16 if mode == "bf16" else F32


def _mm_ap(ap, mode):
    # view an f32 AP as float32r for fast matmul; bf16 tiles pass through
    if mode == "f32r":
        return ap.bitcast(mybir.dt.float32r)
    return ap


def emit(tc, outs, ins, mode=MM_MODE):
    """Emit the per-core program. outs/ins: dicts of DRAM APs."""
    nc = tc.nc
    wdt = _wdt(mode)
    table, idx, val = ins["table"], ins["idx"], ins["val"]
    w1s, w2s, b1s, b2s, ident = ins["w1s"], ins["w2s"], ins["b1s"], ins["b2s"], ins["ident"]
    out_a, out_b = outs["out_a"], outs["out_b"]

    with (
        tc.tile_pool(name="sb", bufs=1) as sb,
        tc.tile_pool(name="pst", bufs=2, space="PSUM") as pst,
        tc.tile_pool(name="psm", bufs=4, space="PSUM") as psm,
        tc.tile_pool(name="psr", bufs=1, space="PSUM") as psr,
    ):
        # ---- constant / input loads (HWDGE; overlap with gather setup) ----
        idx_t = sb.tile([P, G * F], mybir.dt.int32)
        nc.sync.dma_start(out=idx_t[:], in_=idx)
        val_t = sb.tile([P, G * F], F32)
        nc.sync.dma_start(out=val_t[:], in_=val)
        w1_t = sb.tile([P, KP * H], wdt)
        nc.sync.dma_start(out=w1_t[:], in_=w1s)
        w2_t = sb.tile([P, 4 * H], wdt)
        nc.sync.dma_start(out=w2_t[:], in_=w2s)
        b1_t = sb.tile([P, 4], F32)
        nc.sync.dma_start(out=b1_t[:], in_=b1s)
        b2_t = sb.tile([P, 4], F32)
        nc.sync.dma_start(out=b2_t[:], in_=b2s)
        id_t = sb.tile([P, P], F32)
        nc.sync.dma_start(out=id_t[:], in_=ident)
        ones_t = sb.tile([P, 1], wdt)
        nc.vector.memset(ones_t[:], 1.0)

        gath = sb.tile([P, G * F * DP], F32)
        secT = [sb.tile([P, BS], wdt, name=f"secT{c}") for c in range(KP)]
        # last k-chunk has only 112 valid rows; zero so padded rows contribute 0
        nc.vector.memset(secT[KP - 1][:], 0.0)

        # ---- gather: HW indirect DMA consumes ONE index per partition, so
        # each instruction fetches 128 rows (one 68B row per partition).
        # Issue order is feature-chunk-major so each secT k-chunk completes
        # early and layer-1 matmuls accumulate while later chunks gather. ----
        secv = sb.tile([P, G * F * D], F32)
        fstv = sb.tile([P, G * F], F32)
        for c in range(KP):
            f0 = c * 8
            nf = min(F - f0, 8)
            for g in range(G):
                for fi in range(nf):
                    j = g * F + f0 + fi
                    nc.gpsimd.indirect_dma_start(
                        out=gath[:, j * DP:(j + 1) * DP],
                        out_offset=None,
                        in_=table,
                        in_offset=bass.IndirectOffsetOnAxis(
                            ap=idx_t[:, j:j + 1], axis=0
                        ),
                    )
                # scale this (g, chunk) slice by the field value and compact
                gs = gath[:, (g * F + f0) * DP:(g * F + f0 + nf) * DP].rearrange(
                    "p (f d) -> p f d", d=DP
                )
                vs = val_t[:, g * F + f0:g * F + f0 + nf].unsqueeze(2).to_broadcast(
                    [P, nf, D]
                )
                nc.vector.tensor_tensor(
                    out=secv[:, (g * F + f0) * D:(g * F + f0 + nf) * D].rearrange(
                        "p (f d) -> p f d", d=D
                    ),
                    in0=gs[:, :, 0:D],
                    in1=vs,
                    op=mybir.AluOpType.mult,
                )
                # transpose into the k-chunk [FD, batch] layout
                ncol = nf * D
                tp = pst.tile([P, P], F32, tag="tp")
                nc.tensor.transpose(
                    out=tp[:ncol, :],
                    in_=secv[:, (g * F + f0) * D:(g * F + f0 + nf) * D],
                    identity=id_t[:],
                )
                nc.vector.tensor_copy(
                    out=secT[c][0:ncol, g * P:(g + 1) * P], in_=tp[:ncol, :]
                )
        nc.vector.tensor_tensor(
            out=fstv[:].unsqueeze(2),
            in0=gath[:].rearrange("p (j d) -> p j d", d=DP)[:, :, D:DP],
            in1=val_t[:].unsqueeze(2),
            op=mybir.AluOpType.mult,
        )

        # ---- FM sums: per-chunk partials (overlap the gather), then combine --
        s_p = sb.tile([P, KP * G * D], F32)
        q_p = sb.tile([P, KP * G * D], F32)
        sq_t = sb.tile([P, G * F * D], F32)
        for c in range(KP):
            f0 = c * 8
            nf = min(F - f0, 8)
            for g in range(G):
                lo = (g * F + f0) * D
                hi = lo + nf * D
                nc.scalar.square(out=sq_t[:, lo:hi], in_=secv[:, lo:hi])
                nc.vector.reduce_sum(
                    out=s_p[:, (c * G + g) * D:(c * G + g + 1) * D],
                    in_=secv[:, lo:hi].rearrange("p (f d) -> p d f", d=D),
                    axis=mybir.AxisListType.X,
                )
                nc.vector.reduce_sum(
                    out=q_p[:, (c * G + g) * D:(c * G + g + 1) * D],
                    in_=sq_t[:, lo:hi].rearrange("p (f d) -> p d f", d=D),
                    axis=mybir.AxisListType.X,
                )
        s_t = sb.tile([P, G * D], F32)
        q_t = sb.tile([P, G * D], F32)
        # sum the 5 chunk partials (viewed [c, g*d]) down the c axis
        nc.vector.reduce_sum(
            out=s_t[:],
            in_=s_p[:].rearrange("p (c v) -> p v c", c=KP),
            axis=mybir.AxisListType.X,
        )
        nc.vector.reduce_sum(
            out=q_t[:],
            in_=q_p[:].rearrange("p (c v) -> p v c", c=KP),
            axis=mybir.AxisListType.X,
        )
        fst_t = sb.tile([P, G], F32)
        nc.vector.reduce_sum(
            out=fst_t[:],
            in_=fstv[:].rearrange("p (g f) -> p g f", g=G),
            axis=mybir.AxisListType.X,
        )
        ss_t = sb.tile([P, G * D], F32)
        nc.vector.tensor_tensor(
            out=ss_t[:], in0=s_t[:], in1=s_t[:], op=mybir.AluOpType.mult
        )
        nc.vector.tensor_tensor(
            out=ss_t[:], in0=ss_t[:], in1=q_t[:], op=mybir.AluOpType.subtract
        )
        fm_t = sb.tile([P, G], F32)
        nc.vector.reduce_sum(
            out=fm_t[:],
            in_=ss_t[:].rearrange("p (g d) -> p g d", g=G),
            axis=mybir.AxisListType.X,
        )
        oa_t = sb.tile([P, G], F32)
        nc.vector.scalar_tensor_tensor(
            out=oa_t[:],
            in0=fm_t[:],
            scalar=0.5,
            in1=fst_t[:],
            op0=mybir.AluOpType.mult,
            op1=mybir.AluOpType.add,
        )
        nc.sync.dma_start(out=out_a, in_=oa_t[:])

        # ---- DNN in two batch halves so half 0 finishes while the last
        # gather chunk's remaining groups are still in flight ----
        NB = 2
        W = BS // NB
        h_t = [sb.tile([MCH[j], BS], wdt, name=f"h{j}") for j in range(4)]
        r_t = [sb.tile([MCH[j], BS], wdt, name=f"r{j}") for j in range(4)]
        ob_t = sb.tile([1, BS], F32)
        for half in range(NB):
            cs = slice(half * W, (half + 1) * W)
            for j in range(4):
                pm = psm.tile([MCH[j], W], F32, tag="mm", padded_shape=[P, W])
                for c in range(KP):
                    lhsT = w1_t[:, c * H + j * P: c * H + j * P + MCH[j]]
                    nc.tensor.matmul(
                        out=pm[:],
                        lhsT=_mm_ap(lhsT, mode),
                        rhs=_mm_ap(secT[c][:, cs], mode),
                        start=(c == 0),
                        stop=(c == KP - 1),
                    )
                nc.scalar.activation(
                    out=h_t[j][:, cs],
                    in_=pm[:],
                    func=mybir.ActivationFunctionType.Relu,
                    bias=b1_t[0:MCH[j], j:j + 1],
                    scale=1.0,
                )
            for j in range(4):
                pm2 = psm.tile([MCH[j], W], F32, tag="mm", padded_shape=[P, W])
                for c in range(4):
                    lhsT = w2_t[0:MCH[c], c * H + j * P: c * H + j * P + MCH[j]]
                    nc.tensor.matmul(
                        out=pm2[:],
                        lhsT=_mm_ap(lhsT, mode),
                        rhs=_mm_ap(h_t[c][:, cs], mode),
                        start=(c == 0),
                        stop=(c == 3),
                    )
                nc.scalar.activation(
                    out=r_t[j][:, cs],
                    in_=pm2[:],
                    func=mybir.ActivationFunctionType.Relu,
                    bias=b2_t[0:MCH[j], j:j + 1],
                    scale=1.0,
                )
            # deep.sum over H via ones-matmul (partition reduction)
            p3 = psr.tile([1, W], F32, tag="red")
            for j in range(4):
                nc.tensor.matmul(
                    out=p3[:],
                    lhsT=_mm_ap(ones_t[0:MCH[j], 0:1], mode),
                    rhs=_mm_ap(r_t[j][:, cs], mode),
                    start=(j == 0),
                    stop=(j == 3),
                )
            nc.vector.tensor_copy(out=ob_t[0:1, cs], in_=p3[:])
        nc.sync.dma_start(out=out_b, in_=ob_t[:])


def build(mode=MM_MODE):
    """Build + compile the per-core Bass program (cached per mode)."""
    if mode in _CACHE:
        return _CACHE[mode]
    wdt_np = np.dtype(mybir.dt.np(_wdt(mode)))
    nc = bacc.Bacc(
        "TRN2", target_bir_lowering=False, debug=False, num_devices=NCORES
    )
    ins = {
        "table": nc.dram_tensor("table", [F * V, DP], F32, kind="ExternalInput").ap(),
        "idx": nc.dram_tensor("idx", [P, G * F], mybir.dt.int32, kind="ExternalInput").ap(),
        "val": nc.dram_tensor("val", [P, G * F], F32, kind="ExternalInput").ap(),
        "w1s": nc.dram_tensor("w1s", [P, KP * H], mybir.dt.from_np(wdt_np), kind="ExternalInput").ap(),
        "w2s": nc.dram_tensor("w2s", [P, 4 * H], mybir.dt.from_np(wdt_np), kind="ExternalInput").ap(),
        "b1s": nc.dram_tensor("b1s", [P, 4], F32, kind="ExternalInput").ap(),
        "b2s": nc.dram_tensor("b2s", [P, 4], F32, kind="ExternalInput").ap(),
        "ident": nc.dram_tensor("ident", [P, P], F32, kind="ExternalInput").ap(),
    }
    outs = {
        "out_a": nc.dram_tensor("out_a", [P, G], F32, kind="ExternalOutput").ap(),
        "out_b": nc.dram_tensor("out_b", [1, BS], F32, kind="ExternalOutput").ap(),
    }
    with tile.TileContext(nc) as tc:
        emit(tc, outs, ins, mode)
    nc.compile()
    _CACHE[mode] = nc
    return nc


def prep_inputs(inputs, mode=MM_MODE):
    """Host-side packing: returns (in_maps list per core, bias scalar)."""
    wdt_np = np.dtype(mybir.dt.np(_wdt(mode)))
    Xi = np.asarray(inputs["Xi"])[:, :, 0]
    Xv = np.asarray(inputs["Xv"], dtype=np.float32)[:, :, 0]
    sec = np.asarray(inputs["sec_tables"], dtype=np.float32)
    fst = np.asarray(inputs["fst_tables"], dtype=np.float32)
    W1 = np.asarray(inputs["W1"], dtype=np.float32)
    b1 = np.asarray(inputs["b1"], dtype=np.float32)
    W2 = np.asarray(inputs["W2"], dtype=np.float32)
    b2 = np.asarray(inputs["b2"], dtype=np.float32)
    bias = float(np.asarray(inputs["bias"], dtype=np.float32).reshape(-1)[0])

    table = np.concatenate([sec, fst], axis=2).reshape(F * V, DP)
    table = np.ascontiguousarray(table, dtype=np.float32)

    idx_all = (Xi.astype(np.int64) + (np.arange(F, dtype=np.int64) * V)[None, :]).astype(np.int32)

    w1p = np.zeros((FDP, H), np.float32)
    w1p[:FD] = W1
    w1s = np.ascontiguousarray(
        w1p.reshape(KP, P, H).transpose(1, 0, 2).reshape(P, KP * H)
    ).astype(wdt_np)
    w2p = np.zeros((4 * P, H), np.float32)
    w2p[:H] = W2
    w2s = np.ascontiguousarray(
        w2p.reshape(4, P, H).transpose(1, 0, 2).reshape(P, 4 * H)
    ).astype(wdt_np)
    b1p = np.zeros((4 * P,), np.float32)
    b1p[:H] = b1
    b1s = np.ascontiguousarray(b1p.reshape(4, P).T)
    b2p = np.zeros((4 * P,), np.float32)
    b2p[:H] = b2
    b2s = np.ascontiguousarray(b2p.reshape(4, P).T)
    ident = np.eye(P, dtype=np.float32)

    in_maps = []
    for m in range(NCORES):
        sl = slice(m * BS, (m + 1) * BS)
        idx_c = np.ascontiguousarray(
            idx_all[sl].reshape(G, P, F).transpose(1, 0, 2).reshape(P, G * F)
        )
        val_c = np.ascontiguousarray(
            Xv[sl].reshape(G, P, F).transpose(1, 0, 2).reshape(P, G * F)
        )
        in_maps.append(
            dict(
                table=table, idx=idx_c, val=val_c,
                w1s=w1s, w2s=w2s, b1s=b1s, b2s=b2s, ident=ident,
            )
        )
    return in_maps, bias


def assemble(results, bias):
    out = np.empty(B, np.float32)
    for m in range(NCORES):
        oa = np.asarray(results[m]["out_a"], dtype=np.float32)  # [128, 4]
        ob = np.asarray(results[m]["out_b"], dtype=np.float32)  # [1, 512]
        out[m * BS:(m + 1) * BS] = oa.T.reshape(BS) + ob[0] + bias
    return out


def run(inputs, mode=MM_MODE, trace=False, **kwargs):
    nc = build(mode)
    in_maps, bias = prep_inputs(inputs, mode)
    res = run_bass_kernel_spmd(
        nc, in_maps, core_ids=list(range(NCORES)), trace=trace, **kwargs
    )
    return assemble(res.results, bias), res


def kernel(**inputs):
    out, _ = run(inputs)
    return out

